# revision 19
# baseline (speedup 1.0000x reference)
"""PointMixer IntraSetLayer — Trainium2 Bass kernel (8 NeuronCores, SPMD).

Strategy (channel-major):
  * Shard points (rows of p/x/knn_idx) across 8 cores: 12500 points/core,
    padded to 12544 = 98 super-chunks x 128 points.
  * pack = [p | x] rows [N, 67] f32 replicated; per super-chunk one indirect
    DMA gathers 2048 neighbor rows -> G [128 pairs, 16 slots, 67].
  * PE transposes G blocks to channel-major GT [67, 512] and runs a fully
    folded matmul chain (BN folded, bilinear form eigendecomposed into
    projections + squares) down to the softmax weights, then transposes xk
    back to pair-major for the output.
All parameters are folded host-side into a small set of constant matrices.
"""
import sys
from contextlib import ExitStack

import numpy as np

sys.path.insert(0, "/opt/trn_rl_repo")

import concourse.bass as bass  # noqa: E402
import concourse.bacc as bacc  # noqa: E402
import concourse.tile as tile  # noqa: E402
from concourse import mybir  # noqa: E402

EPS = 1e-5
N, K, CIN, COUT, SP = 100000, 16, 64, 64, 8
N_CORES = 8
NLOC = N // N_CORES          # 12500
NSC = (NLOC + 127) // 128    # 98 super-chunks of 128 points
NPAD = NSC * 128             # 12544

F32 = mybir.dt.float32
I32 = mybir.dt.int32

# PE tile_position rules force 32-aligned base partitions for matmul
# operands/outputs, so the fused A tile is laid out as:
#   rows 0:16 h_pre | 32:35 pe_pre | 64:128 xv (+p_embed accumulated)
# and lp2_eff/W_ps are shipped padded so their data sits at base partition 32.
CONST_SHAPES = [
    ("W1", [67, 128]), ("W1neg", [3, 128]), ("bias19", [35, 1]),
    ("Pa", [16, 128]), ("Pb", [16, 128]),
    ("W_ps", [35, 64]), ("lp2_eff", [35, 64]), ("Ma", [128, 64]), ("Mb", [128, 64]),
    ("bias2a", [64, 1]), ("c2b_eff", [64, 8]), ("bias2b", [8, 1]),
    ("c2c_w", [8, 8]), ("c2c_b", [8, 1]), ("bias_v", [64, 1]),
    ("B8", [8, 64]), ("S16", [128, 8]), ("I128", [128, 128]),
]


def fold_params(ip):
    """Fold all BN/bias/bilinear params into the device constant set."""
    f = lambda k: np.asarray(ip[k], np.float32)
    w01, b01 = f("w01"), f("b01")
    blW, blB = f("blW"), f("blB")
    lp1_w, lp1_b = f("lp1_w"), f("lp1_b")
    bnp_g, bnp_b, bnp_m, bnp_v = f("bnp_g"), f("bnp_b"), f("bnp_m"), f("bnp_v")
    lp2_w, lp2_b = f("lp2_w"), f("lp2_b")
    c2a_w = f("c2a_w")
    g2a, b2a, m2a, v2a = f("bn2a_g"), f("bn2a_b"), f("bn2a_m"), f("bn2a_v")
    c2b_w = f("c2b_w")
    g2b, b2b, m2b, v2b = f("bn2b_g"), f("bn2b_b"), f("bn2b_m"), f("bn2b_v")
    c2c_w, c2c_b = f("c2c_w"), f("c2c_b")
    w03, b03 = f("w03"), f("b03")

    s1 = bnp_g / np.sqrt(bnp_v + EPS)
    # GT row order is [x (0:64) | p (64:67)] (pack = [x | p])
    W1 = np.zeros((67, 128), np.float32)
    W1[0:64, 0:16] = w01[3:67]
    W1[64:67, 0:16] = w01[0:3]
    W1[64:67, 32:35] = lp1_w * s1[None, :]
    W1[0:64, 64:128] = w03
    W1neg = -W1[64:67].copy()
    bias19 = np.zeros(35, np.float32)
    bias19[0:16] = b01
    bias19[32:35] = (lp1_b - bnp_m) * s1 + bnp_b
    bias_v = b03 + lp2_b

    Bsym = 0.5 * (blW + np.transpose(blW, (0, 2, 1)))
    lam, vecs = np.linalg.eigh(Bsym.astype(np.float64))
    P = np.zeros((16, 256), np.float32)
    for o in range(16):
        P[:, o * 16:(o + 1) * 16] = vecs[o]
    s2a = g2a / np.sqrt(v2a + EPS)
    c2a_eff = c2a_w * s2a[None, :]
    lp2s = lp2_w.reshape(3, 4, 16).sum(1)
    shrink_bias = lp2_b.reshape(4, 16).sum(0)
    W_ps = lp2s @ c2a_eff[16:32]
    Mfull = lam.astype(np.float32).reshape(256, 1) * np.repeat(c2a_eff[0:16], 16, axis=0)
    bias2a = (b2a - m2a * s2a) + blB @ c2a_eff[0:16] + shrink_bias @ c2a_eff[16:32]
    s2b = g2b / np.sqrt(v2b + EPS)

    B8 = np.zeros((8, 64), np.float32)
    B8[np.arange(64) % 8, np.arange(64)] = 1.0
    S16 = np.zeros((128, 8), np.float32)
    S16[np.arange(128), np.arange(128) // 16] = 1.0

    W_ps_pad = np.zeros((35, 64), np.float32)
    W_ps_pad[32:35] = W_ps
    lp2_pad = np.zeros((35, 64), np.float32)
    lp2_pad[32:35] = lp2_w

    c = {
        "W1": W1, "W1neg": W1neg, "bias19": bias19.reshape(35, 1), "Pa": P[:, :128].copy(),
        "Pb": P[:, 128:].copy(), "W_ps": W_ps_pad, "lp2_eff": lp2_pad,
        "Ma": Mfull[:128].copy(), "Mb": Mfull[128:].copy(),
        "bias2a": bias2a.reshape(64, 1), "c2b_eff": c2b_w * s2b[None, :],
        "bias2b": (b2b - m2b * s2b).reshape(8, 1), "c2c_w": c2c_w,
        "c2c_b": c2c_b.reshape(8, 1), "bias_v": bias_v.reshape(64, 1),
        "B8": B8, "S16": S16, "I128": np.eye(128, dtype=np.float32),
    }
    return {k: np.ascontiguousarray(v, np.float32) for k, v in c.items()}


def build_bass(ntable, nsc):
    """Build the per-core Bass module. nsc = super-chunks (ntable unused —
    neighbor rows are pre-gathered per shard on the host, per the sharding
    strategy "pre-gather neighbor features per shard")."""
    nc = bacc.Bacc("TRN2", target_bir_lowering=False)
    Relu = mybir.ActivationFunctionType.Relu
    Exp = mybir.ActivationFunctionType.Exp
    Square = mybir.ActivationFunctionType.Square
    Ident = mybir.ActivationFunctionType.Identity

    gpack_d = nc.declare_dram_parameter("gpack", [nsc, 128, 16 * 67], F32,
                                        isOutput=False)
    ctr_d = nc.declare_dram_parameter("ctrs", [nsc, 128, 48], F32, isOutput=False)
    ctrcm_d = nc.declare_dram_parameter("ctrs_cm", [nsc, 4, 3, 512], F32, isOutput=False)
    const_d = {n: nc.declare_dram_parameter(n, s, F32, isOutput=False)
               for n, s in CONST_SHAPES}
    npad = nsc * 128
    out_d = nc.declare_dram_parameter("out", [npad, 64], F32, isOutput=True)
    xk_d = nc.declare_dram_parameter("xk", [npad * 16, 64], F32, isOutput=True)
    pr_d = nc.declare_dram_parameter("p_r", [npad * 16, 3], F32, isOutput=True)

    with tile.TileContext(nc) as tc, ExitStack() as ctx:
        cpool = ctx.enter_context(tc.tile_pool(name="consts", bufs=1))
        ct = {}
        for n, s in CONST_SHAPES:
            t = cpool.tile(s, F32, tag=n)
            nc.sync.dma_start(out=t[:], in_=const_d[n][:])
            ct[n] = t

        gp = ctx.enter_context(tc.tile_pool(name="g", bufs=2))
        ctrp = ctx.enter_context(tc.tile_pool(name="ctr", bufs=2))
        ctrcmp = ctx.enter_context(tc.tile_pool(name="ctrcm", bufs=3))
        psubp = ctx.enter_context(tc.tile_pool(name="psub", bufs=2))
        gtp = ctx.enter_context(tc.tile_pool(name="gt", bufs=3))
        hpep = ctx.enter_context(tc.tile_pool(name="hpe", bufs=3))
        sqp = ctx.enter_context(tc.tile_pool(name="sq", bufs=4))
        h2ap = ctx.enter_context(tc.tile_pool(name="h2a", bufs=2))
        h2bp = ctx.enter_context(tc.tile_pool(name="h2b", bufs=2))
        ep = ctx.enter_context(tc.tile_pool(name="e", bufs=2))
        smp = ctx.enter_context(tc.tile_pool(name="sm", bufs=4))
        wsp = ctx.enter_context(tc.tile_pool(name="ws", bufs=2))
        vtp = ctx.enter_context(tc.tile_pool(name="vt", bufs=2))
        xkp = ctx.enter_context(tc.tile_pool(name="xkq", bufs=2))
        xktp = ctx.enter_context(tc.tile_pool(name="xkt", bufs=2))
        outp = ctx.enter_context(tc.tile_pool(name="outs", bufs=2))
        pstags = [("gt", 2), ("a", 1), ("pja", 1), ("pjb", 1),
                  ("hw2", 1), ("cbo", 1), ("xkt", 1)]
        ps = {nm: ctx.enter_context(tc.tile_pool(name="ps_" + nm, bufs=b, space="PSUM"))
              for nm, b in pstags}

        xk_view = xk_d[:].rearrange("(sc s p) c -> sc p s c", s=16, p=128)
        pr_view = pr_d[:].rearrange("(sc s p) c -> sc p s c", s=16, p=128)
        # out rows r = q*32 + b*8 + t  (q = sc*4 + w slice index)
        out_view = out_d[:].rearrange("(q b t) c -> q t b c", b=4, t=8)

        for sc in range(nsc):
            g = gp.tile([128, 16, 67], F32, tag="g")
            nc.sync.dma_start(out=g[:],
                              in_=gpack_d[sc].rearrange("p (s c) -> p s c", c=67))
            ctr = ctrp.tile([128, 48], F32, tag="ctr")
            nc.sync.dma_start(out=ctr[:], in_=ctr_d[sc])
            psub = psubp.tile([128, 48], F32, tag="psub")
            nc.vector.tensor_sub(psub[:].rearrange("p (s c) -> p s c", c=3),
                                 g[:, :, 64:67],
                                 ctr[:].rearrange("p (s c) -> p s c", c=3))
            nc.sync.dma_start(out=pr_view[sc],
                              in_=psub[:].rearrange("p (s c) -> p s c", c=3))

            for w in range(4):
                gt_ps = ps["gt"].tile([67, 512], F32, tag="gt")
                for b in range(4):
                    nc.tensor.transpose(out=gt_ps[:, b * 128:(b + 1) * 128],
                                        in_=g[:, w * 4 + b, :], identity=ct["I128"][:])
                gt = gtp.tile([67, 512], F32, tag="gt")
                nc.vector.tensor_copy(gt[:], gt_ps[:])

                a_ps = ps["a"].tile([128, 512], F32, tag="a")
                nc.tensor.matmul(a_ps[:], lhsT=ct["W1"][:], rhs=gt[:],
                                 start=True, stop=True)
                ctrcm = ctrcmp.tile([3, 512], F32, tag="ctrcm")
                nc.sync.dma_start(out=ctrcm[:], in_=ctrcm_d[sc, w])
                nc.tensor.matmul(a_ps[:], lhsT=ct["W1neg"][:], rhs=ctrcm[:],
                                 start=False, stop=True, skip_group_check=True)
                hpe = hpep.tile([35, 512], F32, tag="hpe")
                nc.scalar.activation(hpe[:], a_ps[0:35, :], Relu,
                                     bias=ct["bias19"][:, 0:1])
                nc.tensor.matmul(a_ps[64:128, :], lhsT=ct["lp2_eff"][32:35, :],
                                 rhs=hpe[32:35, :], start=False, stop=True,
                                 skip_group_check=True)

                vt = vtp.tile([64, 512], F32, tag="vt")
                nc.scalar.activation(vt[:], a_ps[64:128, :], Ident,
                                     bias=ct["bias_v"][:, 0:1])
                hw2 = ps["hw2"].tile([128, 512], F32, tag="hw2")
                h2a_ps = hw2[0:64, :]
                nc.tensor.matmul(h2a_ps, lhsT=ct["W_ps"][32:35, :],
                                 rhs=hpe[32:35, :], start=True, stop=True)
                pja = ps["pja"].tile([128, 512], F32, tag="pja")
                nc.tensor.matmul(pja[:], lhsT=ct["Pa"][:], rhs=hpe[0:16, :],
                                 start=True, stop=True)
                pjb = ps["pjb"].tile([128, 512], F32, tag="pjb")
                nc.tensor.matmul(pjb[:], lhsT=ct["Pb"][:], rhs=hpe[0:16, :],
                                 start=True, stop=True)
                sqa = sqp.tile([128, 512], F32, tag="sq")
                nc.scalar.activation(sqa[:], pja[:], Square)
                sqb = sqp.tile([128, 512], F32, tag="sq")
                nc.scalar.activation(sqb[:], pjb[:], Square)
                nc.tensor.matmul(h2a_ps, lhsT=ct["Ma"][:], rhs=sqa[:],
                                 start=False, stop=True, skip_group_check=True)
                nc.tensor.matmul(h2a_ps, lhsT=ct["Mb"][:], rhs=sqb[:],
                                 start=False, stop=True, skip_group_check=True)
                h2a = h2ap.tile([64, 512], F32, tag="h2a")
                nc.scalar.activation(h2a[:], h2a_ps, Relu, bias=ct["bias2a"][:, 0:1])

                cbo = ps["cbo"].tile([96, 512], F32, tag="cbo")
                nc.tensor.matmul(cbo[0:8, :], lhsT=ct["c2b_eff"][:], rhs=h2a[:],
                                 start=True, stop=True)
                h2b = h2bp.tile([8, 512], F32, tag="h2b")
                nc.scalar.activation(h2b[:], cbo[0:8, :], Relu, bias=ct["bias2b"][:, 0:1])
                nc.tensor.matmul(cbo[32:40, :], lhsT=ct["c2c_w"][:], rhs=h2b[:],
                                 start=True, stop=True)
                et = ep.tile([8, 512], F32, tag="e")
                nc.scalar.activation(et[:], cbo[32:40, :], Exp, bias=ct["c2c_b"][:, 0:1])

                den = smp.tile([8, 32], F32, tag="den")
                nc.vector.reduce_sum(out=den[:],
                                     in_=et[:].rearrange("c (g k) -> c g k", k=16),
                                     axis=mybir.AxisListType.X)
                rec = smp.tile([8, 32], F32, tag="rec")
                nc.vector.reciprocal(rec[:], den[:])
                ws = wsp.tile([8, 512], F32, tag="ws")
                rec_b = bass.AP(tensor=rec.tensor, offset=rec[:].offset,
                                ap=[rec[:].ap[0], rec[:].ap[1], [0, 16]])
                nc.vector.tensor_mul(ws[:].rearrange("c (g k) -> c g k", k=16),
                                     et[:].rearrange("c (g k) -> c g k", k=16),
                                     rec_b)
                wr = hw2[64:128, :]
                nc.tensor.matmul(wr, lhsT=ct["B8"][:], rhs=ws[:],
                                 start=True, stop=True)
                xkq = xkp.tile([64, 512], F32, tag="xkq")
                nc.vector.tensor_mul(xkq[:], wr, vt[:])

                xkt_ps = ps["xkt"].tile([128, 256], F32, tag="xkt")
                for b in range(4):
                    nc.tensor.transpose(out=xkt_ps[:, b * 64:(b + 1) * 64],
                                        in_=xkq[:, b * 128:(b + 1) * 128],
                                        identity=ct["I128"][0:64, 0:64])
                xkt = xktp.tile([128, 256], F32, tag="xkt")
                nc.vector.tensor_copy(xkt[:], xkt_ps[:])
                nc.sync.dma_start(out=xk_view[sc][:, 4 * w:4 * w + 4, :],
                                  in_=xkt[:].rearrange("p (s c) -> p s c", c=64))
                for b in range(4):
                    nc.tensor.matmul(cbo[64:72, b * 64:(b + 1) * 64],
                                     lhsT=ct["S16"][:], rhs=xkt[:, b * 64:(b + 1) * 64],
                                     start=True, stop=True)
                ot = outp.tile([8, 256], F32, tag="ot")
                nc.vector.tensor_copy(ot[:], cbo[64:72, 0:256])
                nc.sync.dma_start(out=out_view[sc * 4 + w],
                                  in_=ot[:].rearrange("t (b c) -> t b c", c=64))
    nc.compile()
    return nc


def prep_core_inputs(pack, knn, p3, consts, nsc):
    """Host-side layout prep for one core's shard.

    knn [npts<=npad, 16] int32 global indices; p3 [npts, 3] center coords.
    """
    npad = nsc * 128
    npts = knn.shape[0]
    if npts < npad:
        knn = np.concatenate([knn, np.zeros((npad - npts, 16), knn.dtype)])
        p3 = np.concatenate([p3, np.zeros((npad - npts, 3), p3.dtype)])
    # idx[sc, p=(t,k), s]: point sc*128 + s*8 + t, k = p%16
    arr = knn.reshape(nsc, 16, 8, 16).transpose(0, 2, 3, 1)
    idxs = arr.reshape(nsc, 128, 16)
    # host-side pre-gather of neighbor rows (sharding hint: "pre-gather
    # neighbor features per shard")
    gpack = pack[idxs.reshape(-1)].reshape(nsc, 128, 16 * 67)
    cc = p3.reshape(nsc, 16, 8, 3)
    cc = np.broadcast_to(cc[:, :, :, None, :], (nsc, 16, 8, 16, 3))
    ctrs4 = cc.transpose(0, 2, 3, 1, 4).reshape(nsc, 128, 16, 3)
    ctrs = np.ascontiguousarray(ctrs4.reshape(nsc, 128, 48), np.float32)
    # channel-major centers per slice: [sc, w, c, (b, p)]
    t = ctrs4.transpose(0, 2, 3, 1)                       # [sc, s, c, p]
    t = t.reshape(nsc, 4, 4, 3, 128).transpose(0, 1, 3, 2, 4)  # [sc, w, c, b, p]
    ctrs_cm = np.ascontiguousarray(t.reshape(nsc, 4, 3, 512), np.float32)
    m = {"gpack": np.ascontiguousarray(gpack, np.float32),
         "ctrs": ctrs, "ctrs_cm": ctrs_cm}
    m.update(consts)
    return m


def unscramble_outputs(res, nsc, npts):
    """Device outputs -> (out, xk, p_r) in reference layout for one core."""
    out = res["out"][:npts]
    # device xk/p_r rows: sc*2048 + s*128 + p where pair = (sc*128 + s*8 + p//16)*16 + p%16
    # = sc*2048 + s*128 + p  => row r IS the global pair index. Direct reshape.
    xk = res["xk"].reshape(nsc * 128, 16, 64)[:npts]
    p_r = res["p_r"].reshape(nsc * 128, 16, 3)[:npts]
    return out, xk, p_r


_CACHE = {}


def _get_nc(ntable, nsc):
    key = (ntable, nsc)
    if key not in _CACHE:
        _CACHE[key] = build_bass(ntable, nsc)
    return _CACHE[key]


def kernel(**inputs):
    from concourse.bass_utils import run_bass_kernel_spmd

    p = np.ascontiguousarray(np.asarray(inputs["p"]), np.float32)
    x = np.ascontiguousarray(np.asarray(inputs["x"]), np.float32)
    knn_in = inputs["knn_idx"]
    knn = np.ascontiguousarray(np.asarray(knn_in), np.int64).astype(np.int32)
    n = p.shape[0]
    assert n == N and knn.shape == (N, K)

    consts = fold_params(inputs)
    pack = np.ascontiguousarray(np.concatenate([x, p], axis=1), np.float32)

    nc = _get_nc(N, NSC)
    in_maps = []
    for c in range(N_CORES):
        rows = slice(c * NLOC, (c + 1) * NLOC)
        in_maps.append(prep_core_inputs(pack, knn[rows], p[rows], consts, NSC))
    res = run_bass_kernel_spmd(nc, in_maps, list(range(N_CORES)))

    outs, xks, prs = [], [], []
    for c in range(N_CORES):
        o, xk_c, pr_c = unscramble_outputs(res.results[c], NSC, NLOC)
        outs.append(o)
        xks.append(xk_c)
        prs.append(pr_c)
    out = np.concatenate(outs, 0)
    xk = np.concatenate(xks, 0)
    p_r = np.concatenate(prs, 0)
    return out, xk, np.asarray(knn_in), p_r
